# revision 4
# baseline (speedup 1.0000x reference)
"""Trainium2 Bass kernel for the dense CNN (CSP block with CARAFE upsamplers).

Strategy: pure data parallelism — 8 samples over 8 NeuronCores, one sample per
core, full forward pass per core:

  cv1 (1x1, 256->256) -> split y / y0
  bneck1 = two 3x3 CBS convs (128->128)        [y1]
  cvm2 (1x1 over pixel_unshuffle(y1,2))        [z2, 64x64]
  carafe(k=1,up=2) == nearest-neighbor 2x upsample + y1
  bneck2                                        [y2]
  cvm3 (1x1 over pixel_unshuffle(y2,4))        [z3, 32x32]
  carafe(k=2,up=4): per-pixel 2x2-tap softmax weights, 4x up + y2  [y3pre]
  bneck3                                        [y3]
  cv2 (1x1, 640->256) over concat(y, y0, y1, y2, y3)

Implementation notes:
- The six 3x3 convs run in fp8(e4m3) DoubleRow mode at 2x PE rate. Accuracy is
  preserved by a two-term weight decomposition: each tap's stationary operand
  is the DR pair (w_hi, w_lo) with w = w_hi + w_lo (both e4m3, per-out-channel
  pow2-prescaled, scale folded into the BN scale), and the moving operand is
  the same fp8 activation read twice (pair-dim stride 0). Weight quant error
  ~2^-8; activation quant error 2^-4 — measured end-to-end rel err ~9e-3.
- fp8 conv inputs live in a "tight" layout [C, 16642]: rows -1..128 at row
  stride 128 (vertical zero pad only) + 1 guard byte at each end, so every
  tap is a flat 512-px DR read. Horizontal wrap garbage in columns 0/127 is
  cancelled exactly by 6 tiny negated-weight DR repair matmuls per psum tile
  (fp8 x fp8 products are exact in the PE's e10m10 pipeline, so the
  subtraction cancels bit-for-bit).
- cv1 / cvm2 / cvm3 / cv2 / the CARAFE chain stay bf16: quantizing the
  cv1-input or the cv2 concat operands lands directly on the output and
  blows the error budget (measured 0.03-0.05).
- y0 is written bf16 (clean copy for cv2) and down-converted to the fp8
  tight map on the otherwise-idle DVE during phase A; y1/y2 stay bf16-only
  (no 3x3 conv reads them). carafe reassembly writes fp8 directly (its
  strided writes run at DVE 1X regardless of dtype).
- cv2 is split as in the bf16 baseline: k-tiles {y,y0,y1,y2} are computed as
  a bf16 partial (acc4, bounced through DRAM) on the PE during the CARAFE DVE
  window; the y3 k-tile + an identity-matmul re-injection of acc4 are chained
  per 4-row tile directly behind bneck3's second conv.
"""
import sys

sys.path.insert(0, "/opt/trn_rl_repo")

import numpy as np
import ml_dtypes

import concourse.bass as bass
import concourse.bacc as bacc
import concourse.mybir as mybir
import concourse.tile as tile

F32 = mybir.dt.float32
BF16 = mybir.dt.bfloat16
F8 = mybir.dt.float8e4
AF = mybir.ActivationFunctionType
DR = mybir.MatmulPerfMode.DoubleRow

N_CORES = 8
C = 128
H = W = 128
HP = H + 2  # padded row length (bf16 maps)
NPIX = H * W
IOFF = HP + 1  # offset of interior (1,1) in padded layout
TLEN = 130 * 128 + 2  # tight fp8 map: guard + rows -1..128 @128 + guard
ALEN = 66 * 128 + 2  # tight split map (66 rows)
ZP = 33  # z3 logical padded side (rows/cols -1..31)
ZS = 34  # z3 row stride (even, for DVE 2x alignment)
TAPS9 = [(dy, dx) for dy in (-1, 0, 1) for dx in (-1, 0, 1)]


def _ap(t, off, dims):
    """Free-dim AP into tile t (keeps full partition dim)."""
    return bass.AP(tensor=t.tensor, offset=t.offset + off, ap=[list(t.ap[0])] + dims)


def _zero_border(nc, mp, side=HP):
    nc.vector.memset(_ap(mp, 0, [[1, side]]), 0.0)
    nc.vector.memset(_ap(mp, (side - 1) * side, [[1, side]]), 0.0)
    nc.vector.memset(_ap(mp, side, [[side, side - 2]]), 0.0)
    nc.vector.memset(_ap(mp, side + side - 1, [[side, side - 2]]), 0.0)


def _zero_tight(nc, mp):
    """Zero front guard + row -1 and row 128 + back guard of a tight map."""
    nc.vector.memset(_ap(mp, 0, [[1, 129]]), 0.0)
    nc.vector.memset(_ap(mp, 129 * 128 + 1, [[1, 129]]), 0.0)


def build_nc():
    nc = bacc.Bacc(None)

    # ---- I/O ----
    x_d = nc.dram_tensor("x", [2, C, NPIX], BF16, kind="ExternalInput")
    w_m = {}
    wn_m = {}
    for name in ("m1a", "m1b", "m2a", "m2b", "m3a", "m3b"):
        w_m[name] = nc.dram_tensor(f"w_{name}", [C, 9, 2, C], F8, kind="ExternalInput")
        wn_m[name] = nc.dram_tensor(f"wn_{name}", [C, 6, 2, C], F8,
                                    kind="ExternalInput")
    w_cv1 = nc.dram_tensor("w_cv1", [2, C, 2 * C], BF16, kind="ExternalInput")
    w_cvm2 = nc.dram_tensor("w_cvm2", [4, C, C], BF16, kind="ExternalInput")
    w_cvm3 = nc.dram_tensor("w_cvm3", [16, C, C], BF16, kind="ExternalInput")
    w_cv2 = nc.dram_tensor("w_cv2", [5, C, 2 * C], BF16, kind="ExternalInput")
    w_dn = nc.dram_tensor("w_dn", [C, 32], BF16, kind="ExternalInput")
    w_en = nc.dram_tensor("w_en", [4, 32, 64], BF16, kind="ExternalInput")
    s64_d = nc.dram_tensor("s64", [64, 64], F32, kind="ExternalInput")
    id_d = nc.dram_tensor("ident", [C, C], BF16, kind="ExternalInput")
    sball_d = nc.dram_tensor("sball", [C, 26], F32, kind="ExternalInput")
    out_d = nc.dram_tensor("out", [2, C, NPIX], F32, kind="ExternalOutput")

    # ---- DRAM scratch ----
    y_sp = nc.dram_tensor("y_sp", [C, NPIX], BF16)  # px-linear
    pad_sp = {k: nc.dram_tensor(f"{k}_sp", [C, HP * HP], BF16) for k in ("y0", "y1")}
    wn_sp = nc.dram_tensor("wn_sp", [64 * 1024], BF16)
    acc4_sp = nc.dram_tensor("acc4_sp", [2, C, NPIX], BF16)  # cv2 partial (y,y0,y1,y2)

    with tile.TileContext(nc) as tc:
        import contextlib

        est = contextlib.ExitStack()
        with est:
            consts = est.enter_context(tc.tile_pool(name="consts", bufs=1))
            # y0p / y1p / y2p: each is dead (spilled/consumed) before the
            # next is written -> one slot
            maps = est.enter_context(tc.tile_pool(name="maps", bufs=1))
            # y0f / y2pp / t3p are never live at the same time -> one slot
            f8maps = est.enter_context(tc.tile_pool(name="f8maps", bufs=1))
            f8aux = est.enter_context(tc.tile_pool(name="f8aux", bufs=1))
            psum = est.enter_context(tc.tile_pool(name="psum", bufs=4, space="PSUM"))
            # cv2-partial stream tiles (y0,y1 padded row-blocks); opened at top
            # level so stages 0-3 can prefetch during bneck2's PE-bound window
            phFpre = est.enter_context(tc.tile_pool(name="phFpre", bufs=3))
            cts = {}

            def load_ct(st):
                ct = phFpre.tile([C, 2 * 2080], BF16, tag="ct4", name="ct4")
                poff = (st * 16 + 1) * HP
                nc.sync.dma_start(out=ct[:, 0:2080],
                                  in_=pad_sp["y0"][:, poff:poff + 2080])
                nc.sync.dma_start(out=ct[:, 2080:2 * 2080],
                                  in_=pad_sp["y1"][:, poff:poff + 2080])
                cts[st] = ct

            # ---- constants (small, plus cv2 weights + identity: persistent) ----
            sball = consts.tile([C, 26], F32, tag="sball", name="sball")
            nc.sync.dma_start(out=sball, in_=sball_d[:, :])
            sb = {}
            off = 0
            for name, w_ in (("cv1", 4), ("cv2", 4), ("m1a", 2), ("m1b", 2),
                             ("m2a", 2), ("m2b", 2), ("m3a", 2), ("m3b", 2),
                             ("cvm2", 2), ("cvm3", 2)):
                sb[name] = sball[:, off:off + w_]
                off += w_
            sb["dn_b"] = sball[0:32, 24:25]
            sb["en_b"] = sball[0:64, 25:26]

            s64 = consts.tile([64, 64], F32, tag="s64")
            nc.sync.dma_start(out=s64, in_=s64_d[:, :])
            wdn = consts.tile([C, 32], BF16, tag="wdn")
            nc.sync.dma_start(out=wdn, in_=w_dn[:, :])
            wen = consts.tile([32, 4, 64], BF16, tag="wen")
            nc.sync.dma_start(out=wen, in_=w_en[:, :, :].rearrange("t p n -> p t n"))
            ident = consts.tile([C, C], BF16, tag="ident")
            nc.sync.dma_start(out=ident, in_=id_d[:, :])
            wcv2 = consts.tile([C, 5, 2 * C], BF16, tag="wcv2")
            nc.sync.dma_start(out=wcv2, in_=w_cv2[:, :, :].rearrange("t p n -> p t n"))

            wsb = {}
            wnb = {}

            def load_w(pool, name, src, shape):
                t = pool.tile(shape, BF16, tag=f"w_{name}", name=f"w_{name}")
                nc.sync.dma_start(out=t, in_=src[:, :, :].rearrange("t p n -> p t n"))
                wsb[name] = t

            def load_w8(pool, name):
                t = pool.tile([C, 9, 2, C], F8, tag=f"w_{name}", name=f"w_{name}")
                nc.sync.dma_start(out=t, in_=w_m[name][:, :, :, :])
                wsb[name] = t
                t2 = pool.tile([C, 6, 2, C], F8, tag=f"wn_{name}", name=f"wn_{name}")
                nc.sync.dma_start(out=t2, in_=wn_m[name][:, :, :, :])
                wnb[name] = t2

            # repair descriptors: (wneg index, dy, psum col, rhs extra offset fn)
            REP = ([(j, dy, 0) for j, dy in enumerate((-1, 0, 1))]
                   + [(3 + j, dy, 127) for j, dy in enumerate((-1, 0, 1))])

            def conv_tile_f8(ps, src_t, src_off, i, wname):
                """Emit the 9 DR tap matmuls + 6 DR repair matmuls for 4-row
                tile i of a tight-layout fp8 source. src_off: tile offset of
                the source's guard byte (0 for full maps; row base handled by
                caller via i)."""
                wt, wn = wsb[wname], wnb[wname]
                for t, (dy, dx) in enumerate(TAPS9):
                    rhs = bass.AP(
                        tensor=src_t.tensor,
                        offset=src_t.offset + src_off + 1 + (4 * i + dy + 1) * 128 + dx,
                        ap=[list(src_t.ap[0]), [0, 2], [1, 512]],
                    )
                    nc.tensor.matmul(ps, wt[:, t, :, :], rhs, start=(t == 0),
                                     stop=False, perf_mode=DR)
                for r, (j, dy, col) in enumerate(REP):
                    if col == 0:
                        roff = src_off + (4 * i + dy + 1) * 128
                    else:
                        roff = src_off + 1 + (4 * i + dy + 2) * 128
                    rhs = bass.AP(
                        tensor=src_t.tensor, offset=src_t.offset + roff,
                        ap=[list(src_t.ap[0]), [0, 2], [128, 4]],
                    )
                    nc.tensor.matmul(_ap(ps, col, [[128, 4]]), wn[:, j, :, :], rhs,
                                     start=False, stop=(r == 5), perf_mode=DR)

            def conv3x3_f8(src, dst, wname, dst_kind):
                """CBS 3x3 in fp8 DR: src tight fp8 map -> dst.
                dst_kind: 'tight' (fp8 flat) or 'padded' (bf16, strided)."""
                s_ap, b_ap = sb[wname][:, 0:1], sb[wname][:, 1:2]
                if dst_kind == "padded":
                    _zero_border(nc, dst)
                else:
                    _zero_tight(nc, dst)
                for i in range(32):
                    ps = psum.tile([C, 512], F32, tag="ps", name="ps")
                    conv_tile_f8(ps, src, 0, i, wname)
                    if dst_kind == "padded":
                        nc.scalar.activation(
                            _ap(dst, IOFF + 4 * i * HP, [[HP, 4], [1, W]]),
                            ps[:, :].rearrange("p (r w) -> p r w", r=4),
                            AF.Silu, bias=b_ap, scale=s_ap,
                        )
                    else:
                        nc.scalar.activation(
                            _ap(dst, 1 + (4 * i + 1) * 128, [[1, 512]]),
                            ps, AF.Silu, bias=b_ap, scale=s_ap,
                        )

            def conv3x3_split_f8(srcA, srcB, dst, wname):
                """CBS 3x3 fp8 DR whose input lives in two tight row-half maps
                (66 rows each: A = rows -1..64, B = rows 63..128); dst is a
                full tight fp8 map."""
                s_ap, b_ap = sb[wname][:, 0:1], sb[wname][:, 1:2]
                _zero_tight(nc, dst)
                for i in range(32):
                    srcm = srcA if i < 16 else srcB
                    ii = i if i < 16 else i - 16
                    ps = psum.tile([C, 512], F32, tag="ps", name="ps")
                    conv_tile_f8(ps, srcm, 0, ii, wname)
                    nc.scalar.activation(
                        _ap(dst, 1 + (4 * i + 1) * 128, [[1, 512]]),
                        ps, AF.Silu, bias=b_ap, scale=s_ap,
                    )

            with tc.tile_pool(name="wearly", bufs=1) as wearly:
                load_w(wearly, "cv1", w_cv1, [C, 2, 2 * C])
                load_w8(wearly, "m1a")

                # ================= Phase A: cv1 =================
                y0p = maps.tile([C, HP * HP], BF16, tag="m")
                _zero_border(nc, y0p)
                y0f = f8maps.tile([C, TLEN], F8, tag="mf8", name="y0f")
                _zero_tight(nc, y0f)
                with tc.tile_pool(name="phA", bufs=3) as phA, \
                     tc.tile_pool(name="phAy", bufs=3) as phAy:
                    for st in range(8):  # stages of 2048 px (16 rows)
                        xt = phA.tile([C, 2, 2048], BF16, tag="xt", name="xt")
                        for kt in range(2):
                            nc.sync.dma_start(
                                out=xt[:, kt, :], in_=x_d[kt, :, st * 2048:(st + 1) * 2048]
                            )
                        yt = phAy.tile([C, 2048], BF16, tag="yt", name="yt")
                        for j in range(4):
                            row0 = 16 * st + 4 * j
                            psy = psum.tile([C, 512], F32, tag="ps", name="ps")
                            psy0 = psum.tile([C, 512], F32, tag="ps", name="ps")
                            for kt in range(2):
                                nc.tensor.matmul(
                                    psy, wsb["cv1"][:, kt, 0:C],
                                    xt[:, kt, j * 512:(j + 1) * 512],
                                    start=(kt == 0), stop=(kt == 1),
                                )
                            for kt in range(2):
                                nc.tensor.matmul(
                                    psy0, wsb["cv1"][:, kt, C:2 * C],
                                    xt[:, kt, j * 512:(j + 1) * 512],
                                    start=(kt == 0), stop=(kt == 1),
                                )
                            nc.scalar.activation(
                                yt[:, j * 512:(j + 1) * 512], psy, AF.Silu,
                                bias=sb["cv1"][:, 2:3], scale=sb["cv1"][:, 0:1],
                            )
                            nc.scalar.activation(
                                _ap(y0p, IOFF + row0 * HP, [[HP, 4], [1, W]]),
                                psy0[:, :].rearrange("p (r w) -> p r w", r=4),
                                AF.Silu, bias=sb["cv1"][:, 3:4], scale=sb["cv1"][:, 1:2],
                            )
                        nc.sync.dma_start(
                            out=y_sp[:, st * 2048:(st + 1) * 2048], in_=yt
                        )
                        # fp8 copy of this 16-row block for m1a (DVE is idle here)
                        nc.vector.tensor_copy(
                            _ap(y0f, 1 + (16 * st + 1) * 128, [[1, 2048]]),
                            _ap(y0p, IOFF + 16 * st * HP, [[HP, 16], [1, W]]),
                        )
                nc.sync.dma_start(out=pad_sp["y0"][:, :], in_=y0p)

                # remaining early weights (loads overlap with phase A/B compute)
                load_w8(wearly, "m1b")
                load_w(wearly, "cvm2", w_cvm2, [C, 4, C])
                load_w8(wearly, "m2a")
                load_w8(wearly, "m2b")

                # ================= Phase B: bneck1 =================
                t1p = f8aux.tile([C, TLEN], F8, tag="auxf8", name="t1p")
                conv3x3_f8(y0f, t1p, "m1a", "tight")
                y1p = maps.tile([C, HP * HP], BF16, tag="m")
                conv3x3_f8(t1p, y1p, "m1b", "padded")
                nc.sync.dma_start(out=pad_sp["y1"][:, :], in_=y1p)
                load_w(consts, "cvm3", w_cvm3, [C, 16, C])

                # ================= Phase C/D: cvm2 + carafe2 =================
                with tc.tile_pool(name="phC", bufs=1) as phC:
                    y2pp = f8maps.tile([C, TLEN], F8, tag="mf8", name="y2pp")
                    _zero_tight(nc, y2pp)
                    for half in range(2):
                        z2 = phC.tile([C, 2048], BF16, tag=f"z2{half}", name="z2")
                        for i in range(4):  # 4 rows of 64 px each per psum tile
                            ps = psum.tile([C, 512], F32, tag="ps", name="ps")
                            for abi in range(4):
                                a, b = abi // 2, abi % 2
                                rhs = _ap(y1p, ((64 * half + 16 * i) + a + 1) * HP + (b + 1),
                                          [[2 * HP, 8], [2, 64]])
                                nc.tensor.matmul(ps, wsb["cvm2"][:, abi, :], rhs,
                                                 start=(abi == 0), stop=(abi == 3))
                            nc.scalar.activation(
                                z2[:, i * 512:(i + 1) * 512], ps, AF.Silu,
                                bias=sb["cvm2"][:, 1:2], scale=sb["cvm2"][:, 0:1],
                            )
                        # carafe2 == NN 2x upsample + y1, for this half's rows;
                        # writes the fp8 tight map m2a reads (strided -> DVE 1X
                        # regardless of dtype)
                        for abi in range(4):
                            a, b = abi // 2, abi % 2
                            nc.vector.tensor_tensor(
                                out=_ap(y2pp, 1 + (64 * half + a + 1) * 128 + b,
                                        [[2 * 128, 32], [2, 64]]),
                                in0=z2[:, :].rearrange("p (h w) -> p h w", h=32),
                                in1=_ap(y1p, IOFF + (64 * half + a) * HP + b,
                                        [[2 * HP, 32], [2, 64]]),
                                op=mybir.AluOpType.add,
                            )

                # ================= Phase E: bneck2 =================
                t2p = f8aux.tile([C, TLEN], F8, tag="auxf8", name="t2p")
                conv3x3_f8(y2pp, t2p, "m2a", "tight")
                for st in range(2):
                    load_ct(st)
                y2p = maps.tile([C, HP * HP], BF16, tag="m")
                conv3x3_f8(t2p, y2p, "m2b", "padded")
                for st in range(2, 4):
                    load_ct(st)

            # late weights (cvm3 was preloaded into consts during phase B/C)
            with tc.tile_pool(name="wlate", bufs=1) as wlate, \
                 tc.tile_pool(name="phFy3", bufs=1) as phFy3:
                load_w8(wlate, "m3a")
                load_w8(wlate, "m3b")

                # ======== Phase F: cvm3 + carafe4 + cv2 partial(y..y2) ========
                with tc.tile_pool(name="phF", bufs=1) as phF, \
                     tc.tile_pool(name="phFwb", bufs=2) as phFwb, \
                     tc.tile_pool(name="phFtmp", bufs=1) as phFtmp, \
                     tc.tile_pool(name="phFacc", bufs=2) as phFacc, \
                     tc.tile_pool(name="phFy", bufs=2) as phFy, \
                     tc.tile_pool(name="phFac", bufs=2) as phFac, \
                     tc.tile_pool(name="psF", bufs=2, space="PSUM") as psF:
                    # z3, padded top/left, row stride 34 (even: DVE 2x alignment)
                    z3a = phF.tile([C, ZS * ZP], BF16, tag="z3a", name="z3a")
                    nc.vector.memset(z3a, 0.0)
                    for i in range(2):  # 16 z3-rows of 32 px per tile
                        ps = psum.tile([C, 512], F32, tag="ps", name="ps")
                        for abi in range(16):
                            a, b = abi // 4, abi % 4
                            rhs = _ap(y2p, (64 * i + a + 1) * HP + (b + 1),
                                      [[4 * HP, 16], [4, 32]])
                            nc.tensor.matmul(ps, wsb["cvm3"][:, abi, :], rhs,
                                             start=(abi == 0), stop=(abi == 15))
                        nc.scalar.activation(
                            _ap(z3a, (16 * i + 1) * ZS + 1, [[ZS, 16], [1, 32]]),
                            ps[:, :].rearrange("p (r w) -> p r w", r=16),
                            AF.Silu, bias=sb["cvm3"][:, 1:2], scale=sb["cvm3"][:, 0:1],
                        )
                    # column-shifted copy: z3b[r, k] = z3a[r, k+1] (even tap starts)
                    z3b = phF.tile([C, ZS * ZP], BF16, tag="z3b", name="z3b")
                    nc.vector.memset(z3b, 0.0)
                    nc.vector.tensor_copy(
                        _ap(z3b, 0, [[ZS, ZP], [1, ZP - 1]]),
                        _ap(z3a, 1, [[ZS, ZP], [1, ZP - 1]]),
                    )

                    # kernel prediction: down 1x1 (128->32), pad(1,0), enc 2x2 (32->64)
                    kt1p = phF.tile([32, ZP * ZP], BF16, tag="kt1p", name="kt1p")
                    _zero_border(nc, kt1p, side=ZP)
                    for i in range(2):
                        ps1 = psF.tile([32, 512], F32, tag="psk", name="psk")
                        nc.tensor.matmul(
                            ps1, wdn, _ap(z3a, (16 * i + 1) * ZS + 1, [[ZS, 16], [1, 32]]),
                            start=True, stop=True,
                        )
                        nc.scalar.activation(
                            _ap(kt1p, (16 * i + 1) * ZP + 1, [[ZP, 16], [1, 32]]),
                            ps1[:, :].rearrange("p (r w) -> p r w", r=16),
                            AF.Identity, bias=sb["dn_b"][:, 0:1],
                        )
                    e_sb = phF.tile([64, 1024], F32, tag="e", name="e_sb")
                    for i in range(2):
                        ps2 = psF.tile([64, 512], F32, tag="psk", name="psk")
                        for t in range(4):
                            di, dj = t // 2, t % 2
                            rhs = bass.AP(
                                tensor=kt1p.tensor,
                                offset=kt1p.offset + (16 * i + di) * ZP + dj,
                                ap=[list(kt1p.ap[0]), [ZP, 16], [1, 32]],
                            )
                            nc.tensor.matmul(ps2, wen[:, t, :], rhs,
                                             start=(t == 0), stop=(t == 3))
                        nc.scalar.activation(
                            e_sb[:, i * 512:(i + 1) * 512],
                            ps2[:, :].rearrange("p (r w) -> p r w", r=16),
                            AF.Exp, bias=sb["en_b"][:, 0:1],
                        )
                    sm = psF.tile([64, 1024], F32, tag="psk", name="psk")
                    for i in range(2):
                        nc.tensor.matmul(sm[:, i * 512:(i + 1) * 512], s64,
                                         e_sb[:, i * 512:(i + 1) * 512],
                                         start=True, stop=True)
                    rden = phF.tile([64, 1024], F32, tag="rden", name="rden")
                    nc.vector.reciprocal(rden, sm)
                    wnt = phF.tile([64, 1024], BF16, tag="wnt", name="wnt")
                    nc.vector.tensor_tensor(out=wnt, in0=e_sb, in1=rden,
                                            op=mybir.AluOpType.mult)
                    nc.sync.dma_start(
                        out=wn_sp[:].rearrange("(p f) -> p f", p=64), in_=wnt)

                    # ---- cv2 partial: acc4 = w_y.y + w_y0.y0 + w_y1.y1 + w_y2.y2
                    # (emitted before the reassembly so these matmuls fill the PE
                    #  while the DVE does the CARAFE products)
                    for st in range(8):
                        if st >= 4:
                            load_ct(st)
                        ct = cts.pop(st)
                        yseg = phFy.tile([C, 2048], BF16, tag="yseg", name="yseg")
                        nc.sync.dma_start(out=yseg,
                                          in_=y_sp[:, st * 2048:(st + 1) * 2048])
                        at = phFac.tile([C, 2, 2048], BF16, tag="at4", name="at4")
                        for j in range(4):
                            row0 = 16 * st + 4 * j
                            for co in range(2):
                                ps = psum.tile([C, 512], F32, tag="ps", name="ps")
                                nc.tensor.matmul(
                                    ps, wcv2[:, 0, co * C:(co + 1) * C],
                                    yseg[:, j * 512:(j + 1) * 512],
                                    start=True, stop=False,
                                )
                                for ki, soff in ((1, 0), (2, 2080)):
                                    nc.tensor.matmul(
                                        ps, wcv2[:, ki, co * C:(co + 1) * C],
                                        _ap(ct, soff + 4 * j * HP + 1, [[HP, 4], [1, W]]),
                                        start=False, stop=False,
                                    )
                                nc.tensor.matmul(
                                    ps, wcv2[:, 3, co * C:(co + 1) * C],
                                    _ap(y2p, IOFF + row0 * HP, [[HP, 4], [1, W]]),
                                    start=False, stop=True,
                                )
                                nc.scalar.copy(at[:, co, j * 512:(j + 1) * 512], ps)
                        for co in range(2):
                            nc.sync.dma_start(
                                out=acc4_sp[co, :, st * 2048:(st + 1) * 2048],
                                in_=at[:, co, :],
                            )

                    # ---- reassembly. Softmax weights sum to 1 over taps, so
                    #   out_s = z00 + sum_{t!=00} w_t*(z_t - z00)
                    # with low-res tap diffs d_t precomputed once (tiny).
                    # Processed by (row-half, quarter q == subpos row r1) into
                    # SPLIT tight fp8 maps (A: rows -1..64, B: rows 63..128) so
                    # bneck3's first conv can start after the first half.
                    dts = {}
                    for t in (1, 2, 3):
                        ti, tj = t // 2, t % 2
                        dt = phF.tile([C, 1024], BF16, tag=f"d{t}", name=f"d{t}")
                        nc.vector.tensor_tensor(
                            out=dt,
                            in0=_ap(z3b if tj else z3a, ti * ZS, [[ZS, 32], [1, 32]]),
                            in1=_ap(z3a, 0, [[ZS, 32], [1, 32]]),
                            op=mybir.AluOpType.subtract,
                        )
                        dts[t] = dt
                    y3A = phFy3.tile([C, ALEN], F8, tag="y3A", name="y3A")
                    y3B = phFy3.tile([C, ALEN], F8, tag="y3B", name="y3B")
                    # A: front guard + row -1; back guard. B: front guard;
                    # row 128 + back guard.
                    nc.vector.memset(_ap(y3A, 0, [[1, 129]]), 0.0)
                    nc.vector.memset(_ap(y3A, ALEN - 1, [[1, 1]]), 0.0)
                    nc.vector.memset(_ap(y3B, 0, [[1, 1]]), 0.0)
                    nc.vector.memset(_ap(y3B, 65 * 128 + 1, [[1, 129]]), 0.0)
                    wn_flat = wn_sp[:]
                    for half, q in ((1, 0), (0, 0), (0, 1), (0, 2), (0, 3),
                                    (1, 1), (1, 2), (1, 3)):
                        if True:
                            hoff = 16 * half
                            accq = phFacc.tile([C, 2048], BF16, tag="accq", name="accq")
                            dst = accq[:, :]
                            tmps = []
                            for t in (1, 2, 3):
                                ti, tj = t // 2, t % 2
                                wb = phFwb.tile([C, 2048], BF16, tag="wb", name="wb")
                                src2 = bass.AP(
                                    tensor=wn_flat.tensor,
                                    offset=wn_flat.offset + t * 16384 + q * 4096
                                    + half * 512,
                                    ap=[[0, C], [1024, 4], [1, 512]],
                                )
                                nc.gpsimd.dma_start(out=wb, in_=src2)
                                dread = _ap(dts[t], half * 512,
                                            [[0, 4], [32, 16], [1, 32]])
                                tmp = phFtmp.tile([C, 2048], BF16, tag=f"tmp{t % 2}",
                                                  name="tmp")
                                nc.vector.tensor_tensor(out=tmp, in0=wb, in1=dread,
                                                        op=mybir.AluOpType.mult)
                                tmps.append(tmp)
                                if t == 2:
                                    nc.vector.tensor_tensor(out=dst, in0=tmps[0],
                                                            in1=tmps[1],
                                                            op=mybir.AluOpType.add)
                            nc.vector.tensor_tensor(out=dst, in0=dst, in1=tmps[2],
                                                    op=mybir.AluOpType.add)
                            nc.vector.tensor_tensor(
                                out=dst, in0=dst,
                                in1=_ap(z3a, (hoff) * ZS, [[0, 4], [ZS, 16], [1, 32]]),
                                op=mybir.AluOpType.add,
                            )
                            # + y2 residual, rows R = 4h+q for h in this half
                            ymap = y3A if half == 0 else y3B
                            pos = [[1, 4], [4 * 128, 16], [4, 32]]
                            nc.vector.tensor_tensor(
                                out=_ap(ymap, 1 + (q + 1) * 128, pos),
                                in0=accq[:, :],
                                in1=_ap(y2p, IOFF + (64 * half + q) * HP,
                                        [[1, 4], [4 * HP, 16], [4, 32]]),
                                op=mybir.AluOpType.add,
                            )
                            # boundary rows shared by both halves
                            if half == 0 and q == 3:  # R=63 -> B row 63
                                nc.vector.tensor_tensor(
                                    out=_ap(y3B, 1, [[1, 4], [4, 32]]),
                                    in0=_ap(accq, 15 * 32, [[512, 4], [1, 32]]),
                                    in1=_ap(y2p, IOFF + 63 * HP, [[1, 4], [4, 32]]),
                                    op=mybir.AluOpType.add,
                                )
                            if half == 1 and q == 0:  # R=64 -> A row 64
                                nc.vector.tensor_tensor(
                                    out=_ap(y3A, 1 + 65 * 128, [[1, 4], [4, 32]]),
                                    in0=_ap(accq, 0, [[512, 4], [1, 32]]),
                                    in1=_ap(y2p, IOFF + 64 * HP, [[1, 4], [4, 32]]),
                                    op=mybir.AluOpType.add,
                                )

                # ===== Phase G: bneck3 + cv2 final, chained per 4-row tile =====
                with tc.tile_pool(name="phGy", bufs=4) as phGy, \
                     tc.tile_pool(name="phGc", bufs=2) as phGc, \
                     tc.tile_pool(name="phGo", bufs=2) as phGo:
                    t3p = f8maps.tile([C, TLEN], F8, tag="mf8", name="t3p")
                    conv3x3_split_f8(y3A, y3B, t3p, "m3a")

                    # m3b produces y3 in 4-row px-linear tiles; cv2 consumes each
                    # immediately (y3 is only ever read by cv2's 1x1 conv).
                    s3, b3 = sb["m3b"][:, 0:1], sb["m3b"][:, 1:2]
                    ca = None
                    ot = None
                    for i in range(32):
                        st, jj = i // 4, i % 4
                        if jj == 0:
                            ca = phGc.tile([C, 2, 2048], BF16, tag="ca", name="ca")
                            nc.sync.dma_start(
                                out=ca, in_=acc4_sp[:, :, st * 2048:(st + 1) * 2048]
                                .rearrange("k p f -> p k f"))
                        if jj % 2 == 0:
                            ot = phGo.tile([C, 2, 1024], F32, tag="ot", name="ot")
                        ps = psum.tile([C, 512], F32, tag="ps", name="ps")
                        conv_tile_f8(ps, t3p, 0, i, "m3b")
                        y3t = phGy.tile([C, 512], BF16, tag="y3t", name="y3t")
                        nc.scalar.activation(y3t, ps, AF.Silu, bias=b3, scale=s3)
                        for co in range(2):
                            ps2 = psum.tile([C, 512], F32, tag="ps", name="ps")
                            nc.tensor.matmul(
                                ps2, ident, ca[:, co, jj * 512:(jj + 1) * 512],
                                start=True, stop=False,
                            )
                            nc.tensor.matmul(
                                ps2, wcv2[:, 4, co * C:(co + 1) * C], y3t,
                                start=False, stop=True,
                            )
                            nc.scalar.activation(
                                ot[:, co, (jj % 2) * 512:(jj % 2 + 1) * 512], ps2,
                                AF.Silu,
                                bias=sb["cv2"][:, 2 + co:3 + co],
                                scale=sb["cv2"][:, co:co + 1],
                            )
                        if jj % 2 == 1:
                            base = st * 2048 + (jj // 2) * 1024
                            for co in range(2):
                                nc.sync.dma_start(
                                    out=out_d[co, :, base:base + 1024],
                                    in_=ot[:, co, :],
                                )
    return nc


def _bf(a):
    return np.ascontiguousarray(a.astype(ml_dtypes.bfloat16))


def _f8(a):
    return np.ascontiguousarray(a.astype(ml_dtypes.float8_e4m3))


def prep_base_inputs(inp):
    """Host-side weight rearrangement -> the flat in_map (minus x)."""
    d = {}

    sball = np.zeros((C, 26), np.float32)
    sb_off = {"cv1": 0, "cv2": 4, "m1a": 8, "m1b": 10, "m2a": 12, "m2b": 14,
              "m3a": 16, "m3b": 18, "cvm2": 20, "cvm3": 22}

    def csb(pre, s, b, ntile):
        # scale/bias packed: cols [o:o+ntile]=scale, [o+ntile:o+2*ntile]=bias
        o = sb_off[pre]
        for i in range(ntile):
            sball[:, o + i] = s[i * C:(i + 1) * C]
            sball[:, o + ntile + i] = b[i * C:(i + 1) * C]

    # cv1: w [256, 256, 1, 1] -> [2 (ci tile), 128, 256 co]
    w = inp["cv1_w"][:, :, 0, 0]  # [co, ci]
    d["w_cv1"] = _bf(w.T.reshape(2, C, 2 * C))
    csb("cv1", inp["cv1_s"], inp["cv1_b"], 2)

    # the six 3x3 convs: per-out-channel pow2 prescale, two-term fp8 split
    # w = (w_hi + w_lo)/sc, packed [ci, tap, {hi,lo}, co]; negated copies of
    # the 6 edge-column taps for the wrap-repair matmuls.
    for name in ("m1a", "m1b", "m2a", "m2b", "m3a", "m3b"):
        w = inp[f"{name}_w"].astype(np.float32)  # [co, ci, 3, 3]
        mx = np.abs(w.reshape(C, -1)).max(axis=1)
        mx[mx == 0] = 1.0
        sc = 2.0 ** np.floor(np.log2(224.0 / mx))
        wsc = w * sc[:, None, None, None]
        w_hi = wsc.astype(ml_dtypes.float8_e4m3)
        w_lo = (wsc - w_hi.astype(np.float32)).astype(ml_dtypes.float8_e4m3)
        pack = np.zeros((C, 9, 2, C), ml_dtypes.float8_e4m3)
        for t, (dy, dx) in enumerate(TAPS9):
            pack[:, t, 0, :] = w_hi[:, :, dy + 1, dx + 1].T
            pack[:, t, 1, :] = w_lo[:, :, dy + 1, dx + 1].T
        d[f"w_{name}"] = np.ascontiguousarray(pack)
        packn = np.zeros((C, 6, 2, C), ml_dtypes.float8_e4m3)
        for j, dy in enumerate((-1, 0, 1)):  # col-0 repairs: taps (dy, -1)
            packn[:, j, 0, :] = (-w_hi[:, :, dy + 1, 0]).T
            packn[:, j, 1, :] = (-w_lo[:, :, dy + 1, 0]).T
        for j, dy in enumerate((-1, 0, 1)):  # col-127 repairs: taps (dy, +1)
            packn[:, 3 + j, 0, :] = (-w_hi[:, :, dy + 1, 2]).T
            packn[:, 3 + j, 1, :] = (-w_lo[:, :, dy + 1, 2]).T
        d[f"wn_{name}"] = np.ascontiguousarray(packn)
        csb(name, inp[f"{name}_s"] / sc, inp[f"{name}_b"], 1)

    w = inp["cvm2_w"][:, :, 0, 0].reshape(C, C, 4)  # [co, c, ab]
    d["w_cvm2"] = _bf(np.transpose(w, (2, 1, 0)))  # [ab, ci, co]
    csb("cvm2", inp["cvm2_s"], inp["cvm2_b"], 1)
    w = inp["cvm3_w"][:, :, 0, 0].reshape(C, C, 16)
    d["w_cvm3"] = _bf(np.transpose(w, (2, 1, 0)))
    csb("cvm3", inp["cvm3_s"], inp["cvm3_b"], 1)
    w = inp["cv2_w"][:, :, 0, 0]  # [256, 640]
    d["w_cv2"] = _bf(w.T.reshape(5, C, 2 * C))
    csb("cv2", inp["cv2_s"], inp["cv2_b"], 2)
    d["w_dn"] = _bf(inp["u3_down_w"][:, :, 0, 0].T)  # [128 ci, 32]
    sball[0:32, 24] = inp["u3_down_b"].astype(np.float32)
    w = inp["u3_enc_w"]  # [64, 32, 2, 2]
    d["w_en"] = _bf(np.transpose(w, (2, 3, 1, 0)).reshape(4, 32, 64))
    sball[0:64, 25] = inp["u3_enc_b"].astype(np.float32)
    d["sball"] = sball
    i_idx = np.arange(64)
    d["s64"] = (i_idx[:, None] % 16 == i_idx[None, :] % 16).astype(np.float32)
    d["ident"] = _bf(np.eye(C, dtype=np.float32))
    return d


_NC_CACHE = {}
_TRACE = False  # test.py can flip this to capture an NTFF profile
_LAST_RESULT = None


def get_nc():
    if "nc" not in _NC_CACHE:
        nc = build_nc()
        nc.finalize()  # Bacc: run wait-splitting/reg-alloc passes before lowering
        _NC_CACHE["nc"] = nc
    return _NC_CACHE["nc"]


def make_in_maps(inputs):
    base = prep_base_inputs(inputs)
    x = inputs["x"]  # [8, 256, 128, 128] f32
    xb = _bf(x.reshape(N_CORES, 2, C, NPIX))
    return [dict(base, x=np.ascontiguousarray(xb[i])) for i in range(N_CORES)]


def kernel(**inputs):
    global _LAST_RESULT
    from concourse.bass_utils import run_bass_kernel_spmd

    nc = get_nc()
    in_maps = make_in_maps(inputs)
    res = run_bass_kernel_spmd(
        nc, in_maps, core_ids=list(range(N_CORES)), trace=_TRACE
    )
    _LAST_RESULT = res
    outs = [res.results[i]["out"].reshape(2 * C, H, W) for i in range(N_CORES)]
    return np.stack(outs).astype(np.float32)
